# revision 48
# baseline (speedup 1.0000x reference)
"""BiLSTM-CRF NLL kernel for 8 Trainium2 NeuronCores.

Strategy (3 SPMD launches, host glue between them):
  L1 "layer0": 8 cores = 2 dirs x 4 batch-quarters (16 seqs/core, one LSTM dir).
     gx = W_ih @ x^T + bias is accumulated DIRECTLY into PSUM (bias enters as a
     K=1 ones-row matmul), in chunks of 16 timesteps (4 banks, double-buffered
     across the 8 banks).  The 256-step recurrent scan then accumulates
     W_hh @ h_{t-1} into the same PSUM region and the sigmoid reads PSUM
     straight.  The c-path matmuls are kc-major so the next step's kc=0 block
     only needs the first half of h; tanh/h-mult are split per hidden chunk so
     the PE restarts while the second half of the tail still runs.
  L2 "layer1": same program shape with K=512 input; host reshards and handles
     the per-sequence reversal of the backward direction.
  L3 "logits+CRF": 8 cores = 8 batch-eighths.  Logits matmul, then the CRF
     partition function as an exp-domain matrix recursion
     a_t = (E^T a_{t-1}) * exp(logit_t), E = exp(trans) in bf16 on the PE,
     renormalization every 16 steps folded into the next step's exp(logit)
     factor (off the critical chain).  Masking is avoided by keeping the whole
     a_t history and extracting column t=len_b-1 per sequence via a host-built
     one-hot mask.  start/end/transition numerator terms are summed on host.

Matmuls run in bf16 (fp32 PSUM accumulate); cell state c is fp32.
"""

import os
import sys

import numpy as np

for _p in ("/opt/trn_rl_repo", "/root/.axon_site/_ro/trn_rl_repo"):
    if _p not in sys.path and os.path.isdir(_p):
        sys.path.insert(0, _p)

import ml_dtypes  # noqa: E402

BF16 = ml_dtypes.bfloat16

B, T, V, E, HD, NT = 64, 256, 50000, 256, 256, 20
NCORES = 8
BL = 16            # sequences per core in L1/L2 (batch quarter)
BC = 8             # sequences per core in L3 (batch eighth)
NTOK = BL * T      # tokens per core in L1/L2
NTOK3 = BC * T     # tokens per core in L3
NJ = 8             # gate tiles of 128 rows (4 gates x 256 HD / 128)
TCH = 16           # timesteps per PSUM gx chunk
NCH = TCH * BL     # tokens per gx chunk (256)
NCHUNKS = T // TCH  # 16
RENORM_EVERY = 8   # CRF renorm interval (Ln on ScalarE only covers [0, 2^64],
                   # so colsums must stay below e^44; 8 steps grow < e^26)
NREN = (T - 1) // RENORM_EVERY   # renorm slots used (t = 8,16,...,248)

# gate order stays pytorch-native (i,f,g,o): the c-path gates (i,f,g) are
# tiles 0..5 (one contiguous sigmoid), o is tiles 6..7 (deferred off the
# critical path). g rows are pre-scaled by 2 so tanh(x) = 2*sig(2x)-1.
_PERM = np.arange(4 * HD)

_CACHE = {}
LAST_RESULTS = []   # BassKernelResults of the launches of the last kernel() call


def _mods():
    import concourse.bass as bass
    import concourse.tile as tile
    from concourse import bacc, mybir
    from concourse.bass_utils import run_bass_kernel_spmd
    return bass, tile, bacc, mybir, run_bass_kernel_spmd


def _install_ntff_shim():
    """Provide antenv.axon_hooks (missing in this image) so that
    run_bass_kernel_spmd(trace=True) can capture NTFF profiles through
    libaxon_pjrt.so. Mirrors trn_agent_boot._ntff_profile_via_ctypes."""
    import sys as _sys
    if "antenv.axon_hooks" in _sys.modules:
        return
    import contextlib
    import ctypes
    import types

    so_path = "/opt/axon/libaxon_pjrt.so"
    mod = types.ModuleType("antenv.axon_hooks")
    _hook_box = [None]

    def set_axon_ntff_profile_hook(h):
        _hook_box[0] = h

    def get_axon_ntff_profile_hook():
        return _hook_box[0]

    mod.set_axon_ntff_profile_hook = set_axon_ntff_profile_hook
    mod.get_axon_ntff_profile_hook = get_axon_ntff_profile_hook
    _sys.modules["antenv.axon_hooks"] = mod

    try:
        lib = ctypes.CDLL(so_path)
        if not hasattr(lib, "axon_start_nrt_profile"):
            return
        lib.axon_start_nrt_profile.argtypes = [
            ctypes.POINTER(ctypes.c_int64), ctypes.c_size_t]
        lib.axon_start_nrt_profile.restype = ctypes.c_int64
        lib.axon_stop_nrt_profile.argtypes = [ctypes.c_char_p]
        lib.axon_stop_nrt_profile.restype = ctypes.c_int64

        @contextlib.contextmanager
        def _hook(output_dir, device_ids):
            import jax
            jax.devices()
            if device_ids:
                ids = (ctypes.c_int64 * len(device_ids))(*device_ids)
                rc = lib.axon_start_nrt_profile(ids, len(device_ids))
            else:
                rc = lib.axon_start_nrt_profile(None, 0)
            if rc != 0:
                raise RuntimeError(f"axon_start_nrt_profile rc={rc}")
            try:
                yield
            finally:
                n = lib.axon_stop_nrt_profile(str(output_dir).encode())
                print(f"profile: {n} file(s) written to {output_dir}",
                      file=sys.stderr)

        set_axon_ntff_profile_hook(_hook)
    except OSError:
        pass


# --------------------------------------------------------------------------
# program builders
# --------------------------------------------------------------------------

def build_layer_program(kc_in):
    """One BiLSTM direction for BL sequences. kc_in = input dim / 128."""
    bass, tile, bacc, mybir, _ = _mods()
    dt = mybir.dt
    AF = mybir.ActivationFunctionType
    AO = mybir.AluOpType

    nc = bacc.Bacc("TRN2", target_bir_lowering=False, debug=False)
    xT = nc.dram_tensor("xT", [kc_in, 128, NTOK], dt.bfloat16, kind="ExternalInput").ap()
    wih = nc.dram_tensor("wih", [kc_in, 128, 4 * HD], dt.bfloat16, kind="ExternalInput").ap()
    whh = nc.dram_tensor("whh", [2, 128, 4 * HD], dt.bfloat16, kind="ExternalInput").ap()
    biasw = nc.dram_tensor("biasw", [2, NJ // 2, 128], dt.bfloat16, kind="ExternalInput").ap()
    selw = nc.dram_tensor("selw", [2, 2 * NCH], dt.bfloat16, kind="ExternalInput").ap()
    hout = nc.dram_tensor("hout", [128, 2, T, BL], dt.bfloat16,
                          kind="ExternalOutput").ap()

    with tile.TileContext(nc) as tc:
        with (
            tc.tile_pool(name="w", bufs=1) as wpool,
            tc.tile_pool(name="big", bufs=1) as big,
            tc.tile_pool(name="xs", bufs=3) as xs,
            tc.tile_pool(name="st", bufs=1) as st,
            tc.tile_pool(name="ew", bufs=4) as ew,
            tc.tile_pool(name="gx", bufs=2, space="PSUM") as gxp,
        ):
            wih_sb = wpool.tile([128, kc_in, 4 * HD], dt.bfloat16)
            whh_sb = wpool.tile([128, 2, 4 * HD], dt.bfloat16)
            bias_sb = wpool.tile([2, NJ // 2, 128], dt.bfloat16)
            sel_sb = wpool.tile([2, 2 * NCH], dt.bfloat16)
            for kc in range(kc_in):
                nc.sync.dma_start(wih_sb[:, kc, :], wih[kc])
            nc.sync.dma_start(bias_sb[:], biasw[:])
            # 0/1 selector so one K=2 matmul broadcasts (bias_even|bias_odd)
            # over a whole PSUM bank
            nc.sync.dma_start(sel_sb[:], selw[:])

            hist = big.tile([128, 2, T + 1, BL], dt.bfloat16)
            cst = st.tile([128, 2, BL], dt.float32)
            nc.vector.memset(hist[:, :, 0, :], 0.0)
            nc.vector.memset(cst[:], 0.0)

            # gx matmuls for one chunk, as a list of single-matmul thunks so
            # the scan can drip them into the PE idle window a couple at a
            # time.  Per PSUM bank (gate-tile pair 2bp, 2bp+1): the K=2
            # bias x selector matmul writes EVERY byte of the bank with
            # start=True (hardware clears has_written per element written, so
            # the bank's first writer must cover it fully), then the input
            # projections accumulate.
            def gx_ops(G, xc):
                ops = []
                for bp in range(NJ // 2):
                    acc2 = G[:, 2 * bp:2 * bp + 2].rearrange(
                        "p j t b -> p (j t b)")
                    ops.append(lambda acc2=acc2, bp=bp: nc.tensor.matmul(
                        acc2, bias_sb[:, bp, :], sel_sb[:],
                        start=True, stop=False, skip_group_check=True))
                    for jj in range(2):
                        j = 2 * bp + jj
                        acc = G[:, j].rearrange("p t b -> p (t b)")
                        for kc in range(kc_in):
                            ops.append(
                                lambda acc=acc, j=j, kc=kc: nc.tensor.matmul(
                                    acc, wih_sb[:, kc, j * 128:(j + 1) * 128],
                                    xc[:, kc, :], start=False, stop=False,
                                    skip_group_check=True))
                return ops

            def load_x(n):
                xc = xs.tile([128, kc_in, NCH], dt.bfloat16, name="xc")
                for kc in range(kc_in):
                    nc.sync.dma_start(xc[:, kc, :],
                                      xT[kc, :, n * NCH:(n + 1) * NCH])
                return xc

            # chunk 0 gx up front; chunk 1 x prefetched before the scan so
            # the in-order Sync queue never parks a needed x load behind an
            # hout store that only fires at a chunk boundary
            xc_cur = load_x(0)
            # h2h weights are first needed at t=0's recurrent matmuls, well
            # after chunk-0 gx; keep their DMA behind the x load
            for kc in range(2):
                nc.sync.dma_start(whh_sb[:, kc, :], whh[kc])
            G_cur = gxp.tile([128, NJ, TCH, BL], dt.float32, name="G")
            for op in gx_ops(G_cur, xc_cur):
                op()
            xc_nxt = load_x(1)

            # scan; cell (g rows pre-scaled by 2 on host):
            #   w = (sig_g' - 0.5) * sig_i ; c = 2w + sig_f*c ; h = sig_o*tanh(c)
            for n in range(NCHUNKS):
                G_nxt = None
                if n + 1 < NCHUNKS:
                    G_nxt = gxp.tile([128, NJ, TCH, BL], dt.float32, name="G")
                    # next chunk's gx: the PSUM WAR clears at a chunk
                    # boundary, where the scheduler would clump all of it
                    # ahead of the boundary-critical recurrent matmuls.
                    # Emit in 8 small groups with modeled ready-time floors
                    # spread across this chunk so each lands in a PE idle
                    # window.  (If a floor overestimates, deps still force
                    # the group before the next chunk's sigmoids.)
                    gxq = gx_ops(G_nxt, xc_nxt)
                    ngroups = 8
                    gsz = -(-len(gxq) // ngroups)
                    for g in range(ngroups):
                        sl = gxq[g * gsz:(g + 1) * gsz]
                        if not sl:
                            break
                        # 2.1us/step tracks the scheduler's internal step
                        # estimate (2.3 drifts the floors ~1 step late per
                        # 3 chunks, pushing late groups onto the boundary)
                        pos = n * TCH + 1 + g * 12.0 / ngroups
                        with tc.tile_wait_until(pos * 2.0e-3):
                            for op in sl:
                                op()
                for tt in range(TCH):
                    t = n * TCH + tt
                    # h2h accumulate into the gx PSUM region; kc-major so the
                    # kc=0 block only needs the first half of h_{t-1}
                    for kc in range(2):
                        for j in range(6):
                            nc.tensor.matmul(
                                G_cur[:, j, tt, :],
                                whh_sb[:, kc, j * 128:(j + 1) * 128],
                                hist[:, kc, t, :], start=False,
                                stop=(kc == 1 and j == 5),
                                skip_group_check=True)
                    # o-gate matmuls live in their own PSUM bank (tiles 6,7)
                    # so the c-path sigmoid is not gated on them
                    for j in (6, 7):
                        for kc in range(2):
                            nc.tensor.matmul(
                                G_cur[:, j, tt, :],
                                whh_sb[:, kc, j * 128:(j + 1) * 128],
                                hist[:, kc, t, :], start=False,
                                stop=(j == 7 and kc == 1),
                                skip_group_check=True)


                    A1 = ew.tile([128, 6, BL], dt.float32, name="A1")
                    nc.scalar.activation(A1[:], G_cur[:, 0:6, tt, :], AF.Sigmoid)
                    A2 = ew.tile([128, 2, BL], dt.float32, name="A2")
                    nc.scalar.activation(A2[:], G_cur[:, 6:8, tt, :], AF.Sigmoid)
                    m1 = ew.tile([128, 2, BL], dt.float32, name="m1")
                    nc.vector.tensor_tensor(m1[:], A1[:, 2:4, :], cst[:],
                                            AO.mult)
                    w = ew.tile([128, 2, BL], dt.float32, name="w")
                    nc.vector.scalar_tensor_tensor(
                        w[:], A1[:, 4:6, :], 0.5, A1[:, 0:2, :],
                        AO.subtract, AO.mult)
                    nc.vector.scalar_tensor_tensor(
                        cst[:], w[:], 2.0, m1[:], AO.mult, AO.add)
                    # tail split per hidden chunk: h chunk 0 lands first so the
                    # next step's kc=0 matmuls restart the PE early
                    Tc = ew.tile([128, 2, BL], dt.float32, name="Tc")
                    nc.scalar.activation(Tc[:, 0:1, :], cst[:, 0:1, :], AF.Tanh)
                    nc.scalar.activation(Tc[:, 1:2, :], cst[:, 1:2, :], AF.Tanh)
                    nc.vector.tensor_tensor(hist[:, 0:1, t + 1, :],
                                            A2[:, 0:1, :], Tc[:, 0:1, :],
                                            AO.mult)
                    nc.vector.tensor_tensor(hist[:, 1:2, t + 1, :],
                                            A2[:, 1:2, :], Tc[:, 1:2, :],
                                            AO.mult)
                if G_nxt is not None:
                    G_cur, xc_cur = G_nxt, xc_nxt
                    if n + 2 < NCHUNKS:
                        xc_nxt = load_x(n + 2)
                # stream finished history out
                t0 = n * TCH
                nc.sync.dma_start(hout[:, :, t0:t0 + TCH, :],
                                  hist[:, :, t0 + 1:t0 + TCH + 1, :])
    nc.compile()
    return nc


def build_crf_program():
    bass, tile, bacc, mybir, _ = _mods()
    dt = mybir.dt
    AF = mybir.ActivationFunctionType
    AO = mybir.AluOpType

    nc = bacc.Bacc("TRN2", target_bir_lowering=False, debug=False)
    hcat = nc.dram_tensor("hcat", [4, 128, NTOK3], dt.bfloat16, kind="ExternalInput").ap()
    linw = nc.dram_tensor("linw", [4, 128, NT], dt.bfloat16, kind="ExternalInput").ap()
    linb = nc.dram_tensor("linb", [NT, 1], dt.float32, kind="ExternalInput").ap()
    etrans = nc.dram_tensor("etrans", [NT, NT], dt.bfloat16, kind="ExternalInput").ap()
    estart = nc.dram_tensor("estart", [NT, 1], dt.float32, kind="ExternalInput").ap()
    eend = nc.dram_tensor("eend", [NT, 1], dt.float32, kind="ExternalInput").ap()
    emitmask = nc.dram_tensor("emitmask", [NT, NTOK3], dt.bfloat16, kind="ExternalInput").ap()
    lastsel = nc.dram_tensor("lastsel", [NT, T, BC], dt.bfloat16, kind="ExternalInput").ap()
    smask = nc.dram_tensor("smask", [1, BC, NREN + 1], dt.float32, kind="ExternalInput").ap()
    part_out = nc.dram_tensor("part_out", [1, BC], dt.float32, kind="ExternalOutput").ap()
    emit_out = nc.dram_tensor("emit_out", [1, 1], dt.float32, kind="ExternalOutput").ap()

    NCH3 = 512
    NCHUNKS3 = NTOK3 // NCH3  # 4

    with tile.TileContext(nc) as tc:
        with (
            tc.tile_pool(name="w", bufs=1) as wpool,
            tc.tile_pool(name="big", bufs=1) as big,
            tc.tile_pool(name="sm", bufs=4) as sm,
            tc.tile_pool(name="pslg", bufs=2, space="PSUM") as pslg,
            tc.tile_pool(name="ps", bufs=2, space="PSUM") as ps,
        ):
            hc_sb = big.tile([128, 4, NTOK3], dt.bfloat16)
            for kc in range(4):
                nc.sync.dma_start(hc_sb[:, kc, :], hcat[kc])
            lw_sb = wpool.tile([128, 4, NT], dt.bfloat16)
            for kc in range(4):
                nc.sync.dma_start(lw_sb[:, kc, :], linw[kc])
            lb_sb = wpool.tile([NT, 1], dt.float32)
            nc.sync.dma_start(lb_sb[:], linb[:])
            et_sb = wpool.tile([NT, NT], dt.bfloat16)
            nc.sync.dma_start(et_sb[:], etrans[:])
            es_sb = wpool.tile([NT, 1], dt.float32)
            nc.sync.dma_start(es_sb[:], estart[:])
            ee_sb = wpool.tile([NT, 1], dt.float32)
            nc.sync.dma_start(ee_sb[:], eend[:])
            em_sb = big.tile([NT, NTOK3], dt.bfloat16)
            nc.sync.dma_start(em_sb[:], emitmask[:])
            ls_sb = big.tile([NT, T, BC], dt.bfloat16)
            nc.sync.dma_start(ls_sb[:], lastsel[:])
            sm_sb = wpool.tile([1, BC, NREN + 1], dt.float32)
            nc.sync.dma_start(sm_sb[:], smask[:])
            ones_bf = wpool.tile([NT, 1], dt.bfloat16)
            nc.vector.memset(ones_bf[:], 1.0)
            ones_f = wpool.tile([NT, 1], dt.float32)
            nc.vector.memset(ones_f[:], 1.0)
            onesrow = wpool.tile([1, NT], dt.bfloat16)
            nc.vector.memset(onesrow[:], 1.0)

            # logits^T [NT, t, b] fp32, and exp(logits) in bf16 for the scan
            logits = big.tile([NT, T, BC], dt.float32)
            for n in range(NCHUNKS3):
                acc = pslg.tile([NT, NCH3], dt.float32, name="lg")
                for kc in range(4):
                    nc.tensor.matmul(acc[:], lw_sb[:, kc, :],
                                     hc_sb[:, kc, n * NCH3:(n + 1) * NCH3],
                                     start=(kc == 0), stop=(kc == 3))
                accv = acc[:].rearrange("p (t b) -> p t b", b=BC)
                nc.vector.tensor_scalar_add(
                    logits[:, n * (NCH3 // BC):(n + 1) * (NCH3 // BC), :],
                    accv, lb_sb[:])
            elog = big.tile([NT, T, BC], dt.bfloat16)
            nc.scalar.activation(elog[:], logits[:], AF.Exp)

            # exp-domain forward recursion, two chains of 4 sequences.
            # a is kept in bf16 (the PE moving operand), t-major so every
            # [NT, NBH] time slice is 8B-aligned; renorm is folded into the
            # next step's elog factor, off the chain.
            NBH = BC // 2
            shist = big.tile([1, BC, NREN + 1], dt.float32)
            nc.vector.memset(shist[:], 1.0)
            ahists = []
            for c in range(2):
                ah = big.tile([NT, T, NBH], dt.bfloat16, name=f"ah{c}")
                nc.vector.tensor_scalar_mul(
                    ah[:, 0, :], elog[:, 0, c * NBH:(c + 1) * NBH], es_sb[:])
                ahists.append(ah)
            for t in range(1, T):
                for c in range(2):
                    ah = ahists[c]
                    bsl = slice(c * NBH, (c + 1) * NBH)
                    y = ps.tile([NT, NBH], dt.float32, name=f"y{c}", bufs=1)
                    nc.tensor.matmul(y[:], et_sb[:], ah[:, t - 1, :],
                                     start=True, stop=True)
                    if t % RENORM_EVERY == 0:
                        # colsum of a_{t-1} -> shist slot; fold 1/s into u_t
                        r = t // RENORM_EVERY - 1
                        ssum = ps.tile([NT, NBH], dt.float32, name=f"aux{c}",
                                       bufs=1)[0:1]
                        nc.tensor.matmul(ssum[:], ones_bf[:], ah[:, t - 1, :],
                                         start=True, stop=True)
                        nc.vector.tensor_copy(shist[:, bsl, r], ssum[:])
                        rinv = sm.tile([1, NBH], dt.bfloat16, name=f"rinv{c}")
                        with nc.allow_low_precision(
                                reason="renorm factor; ln(s) accounting "
                                       "absorbs the bf16 rounding"):
                            nc.vector.reciprocal(rinv[:], ssum[:])
                        rb = ps.tile([NT, NBH], dt.float32, name=f"aux{c}",
                                     bufs=1)
                        nc.tensor.matmul(rb[:], onesrow[:], rinv[:],
                                         start=True, stop=True)
                        u1 = sm.tile([NT, NBH], dt.float32, name=f"u1{c}")
                        nc.vector.tensor_tensor(u1[:], elog[:, t, bsl], rb[:],
                                                AO.mult)
                        nc.vector.tensor_tensor(ah[:, t, :], y[:], u1[:],
                                                AO.mult)
                    else:
                        nc.vector.tensor_tensor(ah[:, t, :], y[:],
                                                elog[:, t, bsl], AO.mult)

            # partition_b = ln(sum_j a[len_b-1, j] * e_end[j]) + sum_r ln(s_rb)
            # (lens >= T//2, so the len-1 one-hot lives in t >= T//2-1)
            T_LO = T // 2 - 1
            alast = sm.tile([NT, BC], dt.float32)
            for c in range(2):
                bsl = slice(c * NBH, (c + 1) * NBH)
                prod = big.tile([NT, T - T_LO, NBH], dt.float32,
                                name=f"prod{c}")
                nc.vector.tensor_tensor(prod[:], ahists[c][:, T_LO:, :],
                                        ls_sb[:, T_LO:, bsl], AO.mult)
                nc.vector.reduce_sum(alast[:, bsl],
                                     prod[:].rearrange("p t b -> p b t"),
                                     axis=mybir.AxisListType.X)
            w2 = sm.tile([NT, BC], dt.float32)
            nc.vector.tensor_scalar_mul(w2[:], alast[:], ee_sb[:])
            fsum = ps.tile([1, BC], dt.float32, name="faux", bufs=1)
            nc.tensor.matmul(fsum[:], ones_f[:], w2[:], start=True, stop=True)
            pln = sm.tile([1, BC], dt.float32)
            nc.scalar.activation(pln[:], fsum[:], AF.Ln)
            slog = sm.tile([1, BC, NREN + 1], dt.float32)
            nc.scalar.activation(slog[:], shist[:], AF.Ln)
            slogm = sm.tile([1, BC, NREN + 1], dt.float32)
            nc.vector.tensor_tensor(slogm[:], slog[:], sm_sb[:], AO.mult)
            zb = sm.tile([1, BC], dt.float32)
            nc.vector.reduce_sum(zb[:], slogm[:], axis=mybir.AxisListType.X)
            pout = sm.tile([1, BC], dt.float32)
            nc.vector.tensor_tensor(pout[:], pln[:], zb[:], AO.add)
            nc.sync.dma_start(part_out[:], pout[:])

            # emission score total
            eprod = big.tile([NT, T, BC], dt.float32)
            nc.vector.tensor_tensor(
                eprod[:], logits[:],
                em_sb[:].rearrange("p (t b) -> p t b", b=BC), AO.mult)
            erow = sm.tile([NT, 1], dt.float32)
            nc.vector.reduce_sum(erow[:], eprod[:], axis=mybir.AxisListType.XY)
            etot = ps.tile([1, 1], dt.float32, name="faux", bufs=1)
            nc.tensor.matmul(etot[:], ones_f[:], erow[:], start=True, stop=True)
            eout = sm.tile([1, 1], dt.float32)
            nc.vector.tensor_copy(eout[:], etot[:])
            nc.sync.dma_start(emit_out[:], eout[:])
    nc.compile()
    return nc


# --------------------------------------------------------------------------
# host-side data prep
# --------------------------------------------------------------------------

def _layer_inputs(xin, w_ih, w_hh, b_ih, b_hh):
    """Per-core input dicts for one layer launch.

    xin: [2, B, T, K] fp32 (xin[1] already reversed+masked)
    w_ih: [2, 4HD, K]; w_hh: [2, 4HD, HD]; b_ih, b_hh: [2, 4HD]
    """
    K = xin.shape[-1]
    kc_in = K // 128
    sel = np.zeros((2, 2 * NCH), BF16)
    sel[0, :NCH] = 1.0
    sel[1, NCH:] = 1.0
    # scale the g-gate rows (post-perm block 3) by 2: tanh(x) = 2*sig(2x)-1
    gscale = np.ones((4 * HD, 1), np.float32)
    gscale[2 * HD:3 * HD] = 2.0
    per_dir = []
    for d in range(2):
        wih_p = w_ih[d][_PERM] * gscale
        whh_p = w_hh[d][_PERM] * gscale
        b_p = (b_ih[d] + b_hh[d])[_PERM] * gscale[:, 0]
        wihT = np.ascontiguousarray(
            wih_p.T.reshape(kc_in, 128, 4 * HD)).astype(BF16)
        whhT = np.ascontiguousarray(
            whh_p.T.reshape(2, 128, 4 * HD)).astype(BF16)
        # biasw[p, bp, :] = bias rows for gate tile j = 2*bp + p
        bs = np.ascontiguousarray(
            b_p.reshape(NJ // 2, 2, 128).transpose(1, 0, 2)).astype(BF16)
        per_dir.append((wihT, whhT, bs))
    maps = []
    for core in range(NCORES):
        d, q = divmod(core, 4)
        xc = xin[d, q * BL:(q + 1) * BL]              # [BL, T, K]
        xT = np.ascontiguousarray(
            xc.transpose(2, 1, 0).reshape(kc_in, 128, T * BL)).astype(BF16)
        wihT, whhT, bs = per_dir[d]
        maps.append({"xT": xT, "wih": wihT, "whh": whhT, "biasw": bs,
                     "selw": sel})
    return maps


def _collect_h(results):
    """per-core 'hout' [128,2,T,BL] bf16 -> h [2, B, T, HD] fp32."""
    h = np.empty((2, B, T, HD), np.float32)
    for core in range(NCORES):
        d, q = divmod(core, 4)
        ho = np.asarray(results[core]["hout"], dtype=np.float32)
        h[d, q * BL:(q + 1) * BL] = ho.transpose(3, 2, 1, 0).reshape(BL, T, HD)
    return h


def _unreverse(h_rev, lens, valid):
    """h_rev[b, s] holds position lens_b-1-s; return h[b, t] (zeros at pad)."""
    t = np.arange(T)
    idx = np.clip(lens[:, None] - 1 - t[None, :], 0, T - 1)
    out = np.take_along_axis(h_rev, idx[:, :, None], axis=1)
    return out * valid[:, :, None]


def kernel(**inputs):
    _, _, _, _, run_bass_kernel_spmd = _mods()
    global LAST_RESULTS
    LAST_RESULTS = []
    trace = bool(int(os.environ.get("KERNEL_TRACE", "0")))
    if trace:
        _install_ntff_shim()

    tokens = np.asarray(inputs["tokens"]).astype(np.int64)
    lens = np.asarray(inputs["lens"]).astype(np.int64)
    labels = np.asarray(inputs["labels"]).astype(np.int64)
    emb = np.asarray(inputs["emb"], dtype=np.float32)
    w_ih = [np.asarray(inputs["w_ih_l0"], np.float32),
            np.asarray(inputs["w_ih_l1"], np.float32)]
    w_hh = [np.asarray(inputs["w_hh_l0"], np.float32),
            np.asarray(inputs["w_hh_l1"], np.float32)]
    b_ih = [np.asarray(inputs["b_ih_l0"], np.float32),
            np.asarray(inputs["b_ih_l1"], np.float32)]
    b_hh = [np.asarray(inputs["b_hh_l0"], np.float32),
            np.asarray(inputs["b_hh_l1"], np.float32)]
    lin_w = np.asarray(inputs["lin_w"], np.float32)
    lin_b = np.asarray(inputs["lin_b"], np.float32)
    trans = np.asarray(inputs["trans"], np.float32)
    start_t = np.asarray(inputs["start_t"], np.float32)
    end_t = np.asarray(inputs["end_t"], np.float32)

    t_ar = np.arange(T)
    valid = (t_ar[None, :] < lens[:, None]).astype(np.float32)
    rev_idx = np.clip(lens[:, None] - 1 - t_ar[None, :], 0, T - 1)

    if "layer0" not in _CACHE:
        _CACHE["layer0"] = build_layer_program(E // 128)
    if "layer1" not in _CACHE:
        _CACHE["layer1"] = build_layer_program(2 * HD // 128)
    if "crf" not in _CACHE:
        _CACHE["crf"] = build_crf_program()

    cores = list(range(NCORES))

    # ---------- launch 1: layer 0 ----------
    x = emb[tokens]
    x_rev = np.take_along_axis(x, rev_idx[:, :, None], axis=1) * valid[:, :, None]
    xin0 = np.stack([x, x_rev])
    res1 = run_bass_kernel_spmd(
        _CACHE["layer0"], _layer_inputs(xin0, w_ih[0], w_hh[0], b_ih[0], b_hh[0]),
        cores, trace=trace)
    LAST_RESULTS.append(res1)
    h0 = _collect_h(res1.results)

    # ---------- launch 2: layer 1 ----------
    h0f = h0[0] * valid[:, :, None]
    h0b = _unreverse(h0[1], lens, valid)
    x1 = np.concatenate([h0f, h0b], axis=-1)
    x1_rev = np.take_along_axis(x1, rev_idx[:, :, None], axis=1) * valid[:, :, None]
    xin1 = np.stack([x1, x1_rev])
    res2 = run_bass_kernel_spmd(
        _CACHE["layer1"], _layer_inputs(xin1, w_ih[1], w_hh[1], b_ih[1], b_hh[1]),
        cores, trace=trace)
    LAST_RESULTS.append(res2)
    h1 = _collect_h(res2.results)

    # ---------- launch 3: logits + CRF ----------
    h1f = h1[0] * valid[:, :, None]
    h1b = _unreverse(h1[1], lens, valid)
    hcat = np.concatenate([h1f, h1b], axis=-1)

    lw = np.ascontiguousarray(lin_w.T.reshape(4, 128, NT)).astype(BF16)
    et = np.exp(trans).astype(BF16)
    es = np.exp(start_t).astype(np.float32)[:, None]
    ee = np.exp(end_t).astype(np.float32)[:, None]
    lb = np.ascontiguousarray(lin_b.astype(np.float32)[:, None])
    maps = []
    for core in range(NCORES):
        bs = slice(core * BC, (core + 1) * BC)
        hc = hcat[bs]
        hcT = np.ascontiguousarray(
            hc.transpose(2, 1, 0).reshape(4, 128, T * BC)).astype(BF16)
        em = np.zeros((NT, T, BC), np.float32)
        lab = labels[bs]
        for bb in range(BC):
            em[lab[bb], np.arange(T), bb] = valid[bs][bb]
        ls = np.zeros((NT, T, BC), np.float32)
        for bb in range(BC):
            ls[:, lens[bs][bb] - 1, bb] = 1.0
        r_idx = np.arange(NREN + 1)
        smk = (RENORM_EVERY * (r_idx[None] + 1)
               <= (lens[bs] - 1)[:, None]).astype(np.float32)[None]
        maps.append({
            "hcat": hcT, "linw": lw, "linb": lb, "etrans": et,
            "estart": es, "eend": ee,
            "emitmask": np.ascontiguousarray(
                em.reshape(NT, T * BC)).astype(BF16),
            "lastsel": np.ascontiguousarray(ls).astype(BF16),
            "smask": np.ascontiguousarray(smk),
        })
    res3 = run_bass_kernel_spmd(_CACHE["crf"], maps, cores, trace=trace)
    LAST_RESULTS.append(res3)

    partition = np.concatenate(
        [np.asarray(r["part_out"])[0] for r in res3.results])
    emit = float(sum(np.asarray(r["emit_out"])[0, 0] for r in res3.results))

    # host-side numerator terms
    first_tag = labels[:, 0]
    last_tag = np.take_along_axis(labels, (lens - 1)[:, None], axis=1)[:, 0]
    tr_sc = float((trans[labels[:, :-1], labels[:, 1:]] * valid[:, 1:]).sum())
    host_num = float(start_t[first_tag].sum()) + tr_sc + float(end_t[last_tag].sum())

    loss = partition.sum() - emit - host_num
    return np.float32(loss)


# revision 49
# speedup vs baseline: 1.0252x; 1.0252x over previous
"""BiLSTM-CRF NLL kernel for 8 Trainium2 NeuronCores.

Strategy (3 SPMD launches, host glue between them):
  L1 "layer0": 8 cores = 2 dirs x 4 batch-quarters (16 seqs/core, one LSTM dir).
     gx = W_ih @ x^T + bias is accumulated DIRECTLY into PSUM (bias enters as a
     K=1 ones-row matmul), in chunks of 16 timesteps (4 banks, double-buffered
     across the 8 banks).  The 256-step recurrent scan then accumulates
     W_hh @ h_{t-1} into the same PSUM region and the sigmoid reads PSUM
     straight.  The c-path matmuls are kc-major so the next step's kc=0 block
     only needs the first half of h; tanh/h-mult are split per hidden chunk so
     the PE restarts while the second half of the tail still runs.
  L2 "layer1": same program shape with K=512 input; host reshards and handles
     the per-sequence reversal of the backward direction.
  L3 "logits+CRF": 8 cores = 8 batch-eighths.  Logits matmul, then the CRF
     partition function as an exp-domain matrix recursion
     a_t = (E^T a_{t-1}) * exp(logit_t), E = exp(trans) in bf16 on the PE,
     renormalization every 16 steps folded into the next step's exp(logit)
     factor (off the critical chain).  Masking is avoided by keeping the whole
     a_t history and extracting column t=len_b-1 per sequence via a host-built
     one-hot mask.  start/end/transition numerator terms are summed on host.

Matmuls run in bf16 (fp32 PSUM accumulate); cell state c is fp32.
"""

import os
import sys

import numpy as np

for _p in ("/opt/trn_rl_repo", "/root/.axon_site/_ro/trn_rl_repo"):
    if _p not in sys.path and os.path.isdir(_p):
        sys.path.insert(0, _p)

import ml_dtypes  # noqa: E402

BF16 = ml_dtypes.bfloat16

B, T, V, E, HD, NT = 64, 256, 50000, 256, 256, 20
NCORES = 8
BL = 16            # sequences per core in L1/L2 (batch quarter)
BC = 8             # sequences per core in L3 (batch eighth)
NTOK = BL * T      # tokens per core in L1/L2
NTOK3 = BC * T     # tokens per core in L3
NJ = 8             # gate tiles of 128 rows (4 gates x 256 HD / 128)
TCH = 16           # timesteps per PSUM gx chunk
NCH = TCH * BL     # tokens per gx chunk (256)
NCHUNKS = T // TCH  # 16
RENORM_EVERY = 8   # CRF renorm interval (Ln on ScalarE only covers [0, 2^64],
                   # so colsums must stay below e^44; 8 steps grow < e^26)
NREN = (T - 1) // RENORM_EVERY   # renorm slots used (t = 8,16,...,248)

# gate order stays pytorch-native (i,f,g,o): the c-path gates (i,f,g) are
# tiles 0..5 (one contiguous sigmoid), o is tiles 6..7 (deferred off the
# critical path). g rows are pre-scaled by 2 so tanh(x) = 2*sig(2x)-1.
_PERM = np.arange(4 * HD)

_CACHE = {}
LAST_RESULTS = []   # BassKernelResults of the launches of the last kernel() call


def _mods():
    import concourse.bass as bass
    import concourse.tile as tile
    from concourse import bacc, mybir
    from concourse.bass_utils import run_bass_kernel_spmd
    return bass, tile, bacc, mybir, run_bass_kernel_spmd


def _install_ntff_shim():
    """Provide antenv.axon_hooks (missing in this image) so that
    run_bass_kernel_spmd(trace=True) can capture NTFF profiles through
    libaxon_pjrt.so. Mirrors trn_agent_boot._ntff_profile_via_ctypes."""
    import sys as _sys
    if "antenv.axon_hooks" in _sys.modules:
        return
    import contextlib
    import ctypes
    import types

    so_path = "/opt/axon/libaxon_pjrt.so"
    mod = types.ModuleType("antenv.axon_hooks")
    _hook_box = [None]

    def set_axon_ntff_profile_hook(h):
        _hook_box[0] = h

    def get_axon_ntff_profile_hook():
        return _hook_box[0]

    mod.set_axon_ntff_profile_hook = set_axon_ntff_profile_hook
    mod.get_axon_ntff_profile_hook = get_axon_ntff_profile_hook
    _sys.modules["antenv.axon_hooks"] = mod

    try:
        lib = ctypes.CDLL(so_path)
        if not hasattr(lib, "axon_start_nrt_profile"):
            return
        lib.axon_start_nrt_profile.argtypes = [
            ctypes.POINTER(ctypes.c_int64), ctypes.c_size_t]
        lib.axon_start_nrt_profile.restype = ctypes.c_int64
        lib.axon_stop_nrt_profile.argtypes = [ctypes.c_char_p]
        lib.axon_stop_nrt_profile.restype = ctypes.c_int64

        @contextlib.contextmanager
        def _hook(output_dir, device_ids):
            import jax
            jax.devices()
            if device_ids:
                ids = (ctypes.c_int64 * len(device_ids))(*device_ids)
                rc = lib.axon_start_nrt_profile(ids, len(device_ids))
            else:
                rc = lib.axon_start_nrt_profile(None, 0)
            if rc != 0:
                raise RuntimeError(f"axon_start_nrt_profile rc={rc}")
            try:
                yield
            finally:
                n = lib.axon_stop_nrt_profile(str(output_dir).encode())
                print(f"profile: {n} file(s) written to {output_dir}",
                      file=sys.stderr)

        set_axon_ntff_profile_hook(_hook)
    except OSError:
        pass


# --------------------------------------------------------------------------
# program builders
# --------------------------------------------------------------------------

def build_layer_program(kc_in):
    """One BiLSTM direction for BL sequences. kc_in = input dim / 128."""
    bass, tile, bacc, mybir, _ = _mods()
    dt = mybir.dt
    AF = mybir.ActivationFunctionType
    AO = mybir.AluOpType

    nc = bacc.Bacc("TRN2", target_bir_lowering=False, debug=False)
    xT = nc.dram_tensor("xT", [kc_in, 128, NTOK], dt.bfloat16, kind="ExternalInput").ap()
    wih = nc.dram_tensor("wih", [kc_in, 128, 4 * HD], dt.bfloat16, kind="ExternalInput").ap()
    whh = nc.dram_tensor("whh", [2, 128, 4 * HD], dt.bfloat16, kind="ExternalInput").ap()
    biasw = nc.dram_tensor("biasw", [2, NJ // 2, 128], dt.bfloat16, kind="ExternalInput").ap()
    selw = nc.dram_tensor("selw", [2, 2 * NCH], dt.bfloat16, kind="ExternalInput").ap()
    hout = nc.dram_tensor("hout", [128, 2, T, BL], dt.bfloat16,
                          kind="ExternalOutput").ap()

    with tile.TileContext(nc) as tc:
        with (
            tc.tile_pool(name="w", bufs=1) as wpool,
            tc.tile_pool(name="big", bufs=1) as big,
            tc.tile_pool(name="xs", bufs=3) as xs,
            tc.tile_pool(name="st", bufs=1) as st,
            tc.tile_pool(name="ew", bufs=4) as ew,
            tc.tile_pool(name="gx", bufs=2, space="PSUM") as gxp,
        ):
            wih_sb = wpool.tile([128, kc_in, 4 * HD], dt.bfloat16)
            whh_sb = wpool.tile([128, 2, 4 * HD], dt.bfloat16)
            bias_sb = wpool.tile([2, NJ // 2, 128], dt.bfloat16)
            sel_sb = wpool.tile([2, 2 * NCH], dt.bfloat16)
            for kc in range(kc_in):
                nc.sync.dma_start(wih_sb[:, kc, :], wih[kc])
            nc.sync.dma_start(bias_sb[:], biasw[:])
            # 0/1 selector so one K=2 matmul broadcasts (bias_even|bias_odd)
            # over a whole PSUM bank
            nc.sync.dma_start(sel_sb[:], selw[:])

            hist = big.tile([128, 2, T + 1, BL], dt.bfloat16)
            cst = st.tile([128, 2, BL], dt.float32)
            nc.vector.memset(hist[:, :, 0, :], 0.0)
            nc.vector.memset(cst[:], 0.0)

            # gx matmuls for one chunk, as a list of single-matmul thunks so
            # the scan can drip them into the PE idle window a couple at a
            # time.  Per PSUM bank (gate-tile pair 2bp, 2bp+1): the K=2
            # bias x selector matmul writes EVERY byte of the bank with
            # start=True (hardware clears has_written per element written, so
            # the bank's first writer must cover it fully), then the input
            # projections accumulate.
            def gx_ops(G, xc):
                ops = []
                for bp in range(NJ // 2):
                    acc2 = G[:, 2 * bp:2 * bp + 2].rearrange(
                        "p j t b -> p (j t b)")
                    ops.append(lambda acc2=acc2, bp=bp: nc.tensor.matmul(
                        acc2, bias_sb[:, bp, :], sel_sb[:],
                        start=True, stop=False, skip_group_check=True))
                    for jj in range(2):
                        j = 2 * bp + jj
                        acc = G[:, j].rearrange("p t b -> p (t b)")
                        for kc in range(kc_in):
                            ops.append(
                                lambda acc=acc, j=j, kc=kc: nc.tensor.matmul(
                                    acc, wih_sb[:, kc, j * 128:(j + 1) * 128],
                                    xc[:, kc, :], start=False, stop=False,
                                    skip_group_check=True))
                return ops

            def load_x(n):
                xc = xs.tile([128, kc_in, NCH], dt.bfloat16, name="xc")
                for kc in range(kc_in):
                    nc.sync.dma_start(xc[:, kc, :],
                                      xT[kc, :, n * NCH:(n + 1) * NCH])
                return xc

            # chunk 0 gx up front; chunk 1 x prefetched before the scan so
            # the in-order Sync queue never parks a needed x load behind an
            # hout store that only fires at a chunk boundary
            xc_cur = load_x(0)
            # h2h weights are first needed at t=0's recurrent matmuls, well
            # after chunk-0 gx; keep their DMA behind the x load
            for kc in range(2):
                nc.sync.dma_start(whh_sb[:, kc, :], whh[kc])
            G_cur = gxp.tile([128, NJ, TCH, BL], dt.float32, name="G")
            for op in gx_ops(G_cur, xc_cur):
                op()
            xc_nxt = load_x(1)

            # scan; cell (g rows pre-scaled by 2 on host):
            #   w = (sig_g' - 0.5) * sig_i ; c = 2w + sig_f*c ; h = sig_o*tanh(c)
            for n in range(NCHUNKS):
                G_nxt = None
                if n + 1 < NCHUNKS:
                    G_nxt = gxp.tile([128, NJ, TCH, BL], dt.float32, name="G")
                    # next chunk's gx: the PSUM WAR clears at a chunk
                    # boundary, where the scheduler would clump all of it
                    # ahead of the boundary-critical recurrent matmuls.
                    # Emit in 8 small groups with modeled ready-time floors
                    # spread across this chunk so each lands in a PE idle
                    # window.  (If a floor overestimates, deps still force
                    # the group before the next chunk's sigmoids.)
                    gxq = gx_ops(G_nxt, xc_nxt)
                    ngroups = 8
                    gsz = -(-len(gxq) // ngroups)
                    for g in range(ngroups):
                        sl = gxq[g * gsz:(g + 1) * gsz]
                        if not sl:
                            break
                        # 2.1us/step tracks the scheduler's internal step
                        # estimate (2.3 drifts the floors ~1 step late per
                        # 3 chunks, pushing late groups onto the boundary)
                        pos = n * TCH + 1 + g * 12.0 / ngroups
                        with tc.tile_wait_until(pos * 2.1e-3):
                            for op in sl:
                                op()
                for tt in range(TCH):
                    t = n * TCH + tt
                    # h2h accumulate into the gx PSUM region; kc-major so the
                    # kc=0 block only needs the first half of h_{t-1}
                    for kc in range(2):
                        for j in range(6):
                            nc.tensor.matmul(
                                G_cur[:, j, tt, :],
                                whh_sb[:, kc, j * 128:(j + 1) * 128],
                                hist[:, kc, t, :], start=False,
                                stop=(kc == 1 and j == 5),
                                skip_group_check=True)
                    # o-gate matmuls live in their own PSUM bank (tiles 6,7)
                    # so the c-path sigmoid is not gated on them
                    for j in (6, 7):
                        for kc in range(2):
                            nc.tensor.matmul(
                                G_cur[:, j, tt, :],
                                whh_sb[:, kc, j * 128:(j + 1) * 128],
                                hist[:, kc, t, :], start=False,
                                stop=(j == 7 and kc == 1),
                                skip_group_check=True)


                    A1 = ew.tile([128, 6, BL], dt.float32, name="A1")
                    nc.scalar.activation(A1[:], G_cur[:, 0:6, tt, :], AF.Sigmoid)
                    A2 = ew.tile([128, 2, BL], dt.float32, name="A2")
                    nc.scalar.activation(A2[:], G_cur[:, 6:8, tt, :], AF.Sigmoid)
                    m1 = ew.tile([128, 2, BL], dt.float32, name="m1")
                    nc.vector.tensor_tensor(m1[:], A1[:, 2:4, :], cst[:],
                                            AO.mult)
                    w = ew.tile([128, 2, BL], dt.float32, name="w")
                    nc.vector.scalar_tensor_tensor(
                        w[:], A1[:, 4:6, :], 0.5, A1[:, 0:2, :],
                        AO.subtract, AO.mult)
                    nc.vector.scalar_tensor_tensor(
                        cst[:], w[:], 2.0, m1[:], AO.mult, AO.add)
                    # tail split per hidden chunk: h chunk 0 lands first so the
                    # next step's kc=0 matmuls restart the PE early
                    Tc = ew.tile([128, 2, BL], dt.float32, name="Tc")
                    nc.scalar.activation(Tc[:, 0:1, :], cst[:, 0:1, :], AF.Tanh)
                    nc.scalar.activation(Tc[:, 1:2, :], cst[:, 1:2, :], AF.Tanh)
                    nc.vector.tensor_tensor(hist[:, 0:1, t + 1, :],
                                            A2[:, 0:1, :], Tc[:, 0:1, :],
                                            AO.mult)
                    nc.vector.tensor_tensor(hist[:, 1:2, t + 1, :],
                                            A2[:, 1:2, :], Tc[:, 1:2, :],
                                            AO.mult)
                if G_nxt is not None:
                    G_cur, xc_cur = G_nxt, xc_nxt
                    if n + 2 < NCHUNKS:
                        xc_nxt = load_x(n + 2)
                # stream finished history out
                t0 = n * TCH
                nc.sync.dma_start(hout[:, :, t0:t0 + TCH, :],
                                  hist[:, :, t0 + 1:t0 + TCH + 1, :])
    nc.compile()
    return nc


def build_crf_program():
    bass, tile, bacc, mybir, _ = _mods()
    dt = mybir.dt
    AF = mybir.ActivationFunctionType
    AO = mybir.AluOpType

    nc = bacc.Bacc("TRN2", target_bir_lowering=False, debug=False)
    hcat = nc.dram_tensor("hcat", [4, 128, NTOK3], dt.bfloat16, kind="ExternalInput").ap()
    linw = nc.dram_tensor("linw", [4, 128, NT], dt.bfloat16, kind="ExternalInput").ap()
    linb = nc.dram_tensor("linb", [NT, 1], dt.float32, kind="ExternalInput").ap()
    etrans = nc.dram_tensor("etrans", [NT, NT], dt.bfloat16, kind="ExternalInput").ap()
    estart = nc.dram_tensor("estart", [NT, 1], dt.float32, kind="ExternalInput").ap()
    eend = nc.dram_tensor("eend", [NT, 1], dt.float32, kind="ExternalInput").ap()
    emitmask = nc.dram_tensor("emitmask", [NT, NTOK3], dt.bfloat16, kind="ExternalInput").ap()
    lastsel = nc.dram_tensor("lastsel", [NT, T, BC], dt.bfloat16, kind="ExternalInput").ap()
    smask = nc.dram_tensor("smask", [1, BC, NREN + 1], dt.float32, kind="ExternalInput").ap()
    part_out = nc.dram_tensor("part_out", [1, BC], dt.float32, kind="ExternalOutput").ap()
    emit_out = nc.dram_tensor("emit_out", [1, 1], dt.float32, kind="ExternalOutput").ap()

    NCH3 = 512
    NCHUNKS3 = NTOK3 // NCH3  # 4

    with tile.TileContext(nc) as tc:
        with (
            tc.tile_pool(name="w", bufs=1) as wpool,
            tc.tile_pool(name="big", bufs=1) as big,
            tc.tile_pool(name="sm", bufs=4) as sm,
            tc.tile_pool(name="pslg", bufs=2, space="PSUM") as pslg,
            tc.tile_pool(name="ps", bufs=2, space="PSUM") as ps,
        ):
            hc_sb = big.tile([128, 4, NTOK3], dt.bfloat16)
            for kc in range(4):
                nc.sync.dma_start(hc_sb[:, kc, :], hcat[kc])
            lw_sb = wpool.tile([128, 4, NT], dt.bfloat16)
            for kc in range(4):
                nc.sync.dma_start(lw_sb[:, kc, :], linw[kc])
            lb_sb = wpool.tile([NT, 1], dt.float32)
            nc.sync.dma_start(lb_sb[:], linb[:])
            et_sb = wpool.tile([NT, NT], dt.bfloat16)
            nc.sync.dma_start(et_sb[:], etrans[:])
            es_sb = wpool.tile([NT, 1], dt.float32)
            nc.sync.dma_start(es_sb[:], estart[:])
            ee_sb = wpool.tile([NT, 1], dt.float32)
            nc.sync.dma_start(ee_sb[:], eend[:])
            em_sb = big.tile([NT, NTOK3], dt.bfloat16)
            nc.sync.dma_start(em_sb[:], emitmask[:])
            ls_sb = big.tile([NT, T, BC], dt.bfloat16)
            nc.sync.dma_start(ls_sb[:], lastsel[:])
            sm_sb = wpool.tile([1, BC, NREN + 1], dt.float32)
            nc.sync.dma_start(sm_sb[:], smask[:])
            ones_bf = wpool.tile([NT, 1], dt.bfloat16)
            nc.vector.memset(ones_bf[:], 1.0)
            ones_f = wpool.tile([NT, 1], dt.float32)
            nc.vector.memset(ones_f[:], 1.0)
            onesrow = wpool.tile([1, NT], dt.bfloat16)
            nc.vector.memset(onesrow[:], 1.0)

            # logits^T [NT, t, b] fp32, and exp(logits) in bf16 for the scan
            logits = big.tile([NT, T, BC], dt.float32)
            for n in range(NCHUNKS3):
                acc = pslg.tile([NT, NCH3], dt.float32, name="lg")
                for kc in range(4):
                    nc.tensor.matmul(acc[:], lw_sb[:, kc, :],
                                     hc_sb[:, kc, n * NCH3:(n + 1) * NCH3],
                                     start=(kc == 0), stop=(kc == 3))
                accv = acc[:].rearrange("p (t b) -> p t b", b=BC)
                nc.vector.tensor_scalar_add(
                    logits[:, n * (NCH3 // BC):(n + 1) * (NCH3 // BC), :],
                    accv, lb_sb[:])
            elog = big.tile([NT, T, BC], dt.bfloat16)
            nc.scalar.activation(elog[:], logits[:], AF.Exp)

            # exp-domain forward recursion, two chains of 4 sequences.
            # a is kept in bf16 (the PE moving operand), t-major so every
            # [NT, NBH] time slice is 8B-aligned; renorm is folded into the
            # next step's elog factor, off the chain.
            NBH = BC // 2
            shist = big.tile([1, BC, NREN + 1], dt.float32)
            nc.vector.memset(shist[:], 1.0)
            ahists = []
            for c in range(2):
                ah = big.tile([NT, T, NBH], dt.bfloat16, name=f"ah{c}")
                nc.vector.tensor_scalar_mul(
                    ah[:, 0, :], elog[:, 0, c * NBH:(c + 1) * NBH], es_sb[:])
                ahists.append(ah)
            for t in range(1, T):
                for c in range(2):
                    ah = ahists[c]
                    bsl = slice(c * NBH, (c + 1) * NBH)
                    y = ps.tile([NT, NBH], dt.float32, name=f"y{c}", bufs=1)
                    nc.tensor.matmul(y[:], et_sb[:], ah[:, t - 1, :],
                                     start=True, stop=True)
                    if t % RENORM_EVERY == 0:
                        # colsum of a_{t-1} -> shist slot; fold 1/s into u_t
                        r = t // RENORM_EVERY - 1
                        ssum = ps.tile([NT, NBH], dt.float32, name=f"aux{c}",
                                       bufs=1)[0:1]
                        nc.tensor.matmul(ssum[:], ones_bf[:], ah[:, t - 1, :],
                                         start=True, stop=True)
                        nc.vector.tensor_copy(shist[:, bsl, r], ssum[:])
                        rinv = sm.tile([1, NBH], dt.bfloat16, name=f"rinv{c}")
                        with nc.allow_low_precision(
                                reason="renorm factor; ln(s) accounting "
                                       "absorbs the bf16 rounding"):
                            nc.vector.reciprocal(rinv[:], ssum[:])
                        rb = ps.tile([NT, NBH], dt.float32, name=f"aux{c}",
                                     bufs=1)
                        nc.tensor.matmul(rb[:], onesrow[:], rinv[:],
                                         start=True, stop=True)
                        u1 = sm.tile([NT, NBH], dt.float32, name=f"u1{c}")
                        nc.vector.tensor_tensor(u1[:], elog[:, t, bsl], rb[:],
                                                AO.mult)
                        nc.vector.tensor_tensor(ah[:, t, :], y[:], u1[:],
                                                AO.mult)
                    else:
                        nc.vector.tensor_tensor(ah[:, t, :], y[:],
                                                elog[:, t, bsl], AO.mult)

            # partition_b = ln(sum_j a[len_b-1, j] * e_end[j]) + sum_r ln(s_rb)
            # (lens >= T//2, so the len-1 one-hot lives in t >= T//2-1)
            T_LO = T // 2 - 1
            alast = sm.tile([NT, BC], dt.float32)
            for c in range(2):
                bsl = slice(c * NBH, (c + 1) * NBH)
                prod = big.tile([NT, T - T_LO, NBH], dt.float32,
                                name=f"prod{c}")
                nc.vector.tensor_tensor(prod[:], ahists[c][:, T_LO:, :],
                                        ls_sb[:, T_LO:, bsl], AO.mult)
                nc.vector.reduce_sum(alast[:, bsl],
                                     prod[:].rearrange("p t b -> p b t"),
                                     axis=mybir.AxisListType.X)
            w2 = sm.tile([NT, BC], dt.float32)
            nc.vector.tensor_scalar_mul(w2[:], alast[:], ee_sb[:])
            fsum = ps.tile([1, BC], dt.float32, name="faux", bufs=1)
            nc.tensor.matmul(fsum[:], ones_f[:], w2[:], start=True, stop=True)
            pln = sm.tile([1, BC], dt.float32)
            nc.scalar.activation(pln[:], fsum[:], AF.Ln)
            slog = sm.tile([1, BC, NREN + 1], dt.float32)
            nc.scalar.activation(slog[:], shist[:], AF.Ln)
            slogm = sm.tile([1, BC, NREN + 1], dt.float32)
            nc.vector.tensor_tensor(slogm[:], slog[:], sm_sb[:], AO.mult)
            zb = sm.tile([1, BC], dt.float32)
            nc.vector.reduce_sum(zb[:], slogm[:], axis=mybir.AxisListType.X)
            pout = sm.tile([1, BC], dt.float32)
            nc.vector.tensor_tensor(pout[:], pln[:], zb[:], AO.add)
            nc.sync.dma_start(part_out[:], pout[:])

            # emission score total
            eprod = big.tile([NT, T, BC], dt.float32)
            nc.vector.tensor_tensor(
                eprod[:], logits[:],
                em_sb[:].rearrange("p (t b) -> p t b", b=BC), AO.mult)
            erow = sm.tile([NT, 1], dt.float32)
            nc.vector.reduce_sum(erow[:], eprod[:], axis=mybir.AxisListType.XY)
            etot = ps.tile([1, 1], dt.float32, name="faux", bufs=1)
            nc.tensor.matmul(etot[:], ones_f[:], erow[:], start=True, stop=True)
            eout = sm.tile([1, 1], dt.float32)
            nc.vector.tensor_copy(eout[:], etot[:])
            nc.sync.dma_start(emit_out[:], eout[:])
    nc.compile()
    return nc


# --------------------------------------------------------------------------
# host-side data prep
# --------------------------------------------------------------------------

def _layer_inputs(xin, w_ih, w_hh, b_ih, b_hh):
    """Per-core input dicts for one layer launch.

    xin: [2, B, T, K] fp32 (xin[1] already reversed+masked)
    w_ih: [2, 4HD, K]; w_hh: [2, 4HD, HD]; b_ih, b_hh: [2, 4HD]
    """
    K = xin.shape[-1]
    kc_in = K // 128
    sel = np.zeros((2, 2 * NCH), BF16)
    sel[0, :NCH] = 1.0
    sel[1, NCH:] = 1.0
    # scale the g-gate rows (post-perm block 3) by 2: tanh(x) = 2*sig(2x)-1
    gscale = np.ones((4 * HD, 1), np.float32)
    gscale[2 * HD:3 * HD] = 2.0
    per_dir = []
    for d in range(2):
        wih_p = w_ih[d][_PERM] * gscale
        whh_p = w_hh[d][_PERM] * gscale
        b_p = (b_ih[d] + b_hh[d])[_PERM] * gscale[:, 0]
        wihT = np.ascontiguousarray(
            wih_p.T.reshape(kc_in, 128, 4 * HD)).astype(BF16)
        whhT = np.ascontiguousarray(
            whh_p.T.reshape(2, 128, 4 * HD)).astype(BF16)
        # biasw[p, bp, :] = bias rows for gate tile j = 2*bp + p
        bs = np.ascontiguousarray(
            b_p.reshape(NJ // 2, 2, 128).transpose(1, 0, 2)).astype(BF16)
        per_dir.append((wihT, whhT, bs))
    maps = []
    for core in range(NCORES):
        d, q = divmod(core, 4)
        xc = xin[d, q * BL:(q + 1) * BL]              # [BL, T, K]
        xT = np.ascontiguousarray(
            xc.transpose(2, 1, 0).reshape(kc_in, 128, T * BL)).astype(BF16)
        wihT, whhT, bs = per_dir[d]
        maps.append({"xT": xT, "wih": wihT, "whh": whhT, "biasw": bs,
                     "selw": sel})
    return maps


def _collect_h(results):
    """per-core 'hout' [128,2,T,BL] bf16 -> h [2, B, T, HD] fp32."""
    h = np.empty((2, B, T, HD), np.float32)
    for core in range(NCORES):
        d, q = divmod(core, 4)
        ho = np.asarray(results[core]["hout"], dtype=np.float32)
        h[d, q * BL:(q + 1) * BL] = ho.transpose(3, 2, 1, 0).reshape(BL, T, HD)
    return h


def _unreverse(h_rev, lens, valid):
    """h_rev[b, s] holds position lens_b-1-s; return h[b, t] (zeros at pad)."""
    t = np.arange(T)
    idx = np.clip(lens[:, None] - 1 - t[None, :], 0, T - 1)
    out = np.take_along_axis(h_rev, idx[:, :, None], axis=1)
    return out * valid[:, :, None]


def kernel(**inputs):
    _, _, _, _, run_bass_kernel_spmd = _mods()
    global LAST_RESULTS
    LAST_RESULTS = []
    trace = bool(int(os.environ.get("KERNEL_TRACE", "0")))
    if trace:
        _install_ntff_shim()

    tokens = np.asarray(inputs["tokens"]).astype(np.int64)
    lens = np.asarray(inputs["lens"]).astype(np.int64)
    labels = np.asarray(inputs["labels"]).astype(np.int64)
    emb = np.asarray(inputs["emb"], dtype=np.float32)
    w_ih = [np.asarray(inputs["w_ih_l0"], np.float32),
            np.asarray(inputs["w_ih_l1"], np.float32)]
    w_hh = [np.asarray(inputs["w_hh_l0"], np.float32),
            np.asarray(inputs["w_hh_l1"], np.float32)]
    b_ih = [np.asarray(inputs["b_ih_l0"], np.float32),
            np.asarray(inputs["b_ih_l1"], np.float32)]
    b_hh = [np.asarray(inputs["b_hh_l0"], np.float32),
            np.asarray(inputs["b_hh_l1"], np.float32)]
    lin_w = np.asarray(inputs["lin_w"], np.float32)
    lin_b = np.asarray(inputs["lin_b"], np.float32)
    trans = np.asarray(inputs["trans"], np.float32)
    start_t = np.asarray(inputs["start_t"], np.float32)
    end_t = np.asarray(inputs["end_t"], np.float32)

    t_ar = np.arange(T)
    valid = (t_ar[None, :] < lens[:, None]).astype(np.float32)
    rev_idx = np.clip(lens[:, None] - 1 - t_ar[None, :], 0, T - 1)

    if "layer0" not in _CACHE:
        _CACHE["layer0"] = build_layer_program(E // 128)
    if "layer1" not in _CACHE:
        _CACHE["layer1"] = build_layer_program(2 * HD // 128)
    if "crf" not in _CACHE:
        _CACHE["crf"] = build_crf_program()

    cores = list(range(NCORES))

    # ---------- launch 1: layer 0 ----------
    x = emb[tokens]
    x_rev = np.take_along_axis(x, rev_idx[:, :, None], axis=1) * valid[:, :, None]
    xin0 = np.stack([x, x_rev])
    res1 = run_bass_kernel_spmd(
        _CACHE["layer0"], _layer_inputs(xin0, w_ih[0], w_hh[0], b_ih[0], b_hh[0]),
        cores, trace=trace)
    LAST_RESULTS.append(res1)
    h0 = _collect_h(res1.results)

    # ---------- launch 2: layer 1 ----------
    h0f = h0[0] * valid[:, :, None]
    h0b = _unreverse(h0[1], lens, valid)
    x1 = np.concatenate([h0f, h0b], axis=-1)
    x1_rev = np.take_along_axis(x1, rev_idx[:, :, None], axis=1) * valid[:, :, None]
    xin1 = np.stack([x1, x1_rev])
    res2 = run_bass_kernel_spmd(
        _CACHE["layer1"], _layer_inputs(xin1, w_ih[1], w_hh[1], b_ih[1], b_hh[1]),
        cores, trace=trace)
    LAST_RESULTS.append(res2)
    h1 = _collect_h(res2.results)

    # ---------- launch 3: logits + CRF ----------
    h1f = h1[0] * valid[:, :, None]
    h1b = _unreverse(h1[1], lens, valid)
    hcat = np.concatenate([h1f, h1b], axis=-1)

    lw = np.ascontiguousarray(lin_w.T.reshape(4, 128, NT)).astype(BF16)
    et = np.exp(trans).astype(BF16)
    es = np.exp(start_t).astype(np.float32)[:, None]
    ee = np.exp(end_t).astype(np.float32)[:, None]
    lb = np.ascontiguousarray(lin_b.astype(np.float32)[:, None])
    maps = []
    for core in range(NCORES):
        bs = slice(core * BC, (core + 1) * BC)
        hc = hcat[bs]
        hcT = np.ascontiguousarray(
            hc.transpose(2, 1, 0).reshape(4, 128, T * BC)).astype(BF16)
        em = np.zeros((NT, T, BC), np.float32)
        lab = labels[bs]
        for bb in range(BC):
            em[lab[bb], np.arange(T), bb] = valid[bs][bb]
        ls = np.zeros((NT, T, BC), np.float32)
        for bb in range(BC):
            ls[:, lens[bs][bb] - 1, bb] = 1.0
        r_idx = np.arange(NREN + 1)
        smk = (RENORM_EVERY * (r_idx[None] + 1)
               <= (lens[bs] - 1)[:, None]).astype(np.float32)[None]
        maps.append({
            "hcat": hcT, "linw": lw, "linb": lb, "etrans": et,
            "estart": es, "eend": ee,
            "emitmask": np.ascontiguousarray(
                em.reshape(NT, T * BC)).astype(BF16),
            "lastsel": np.ascontiguousarray(ls).astype(BF16),
            "smask": np.ascontiguousarray(smk),
        })
    res3 = run_bass_kernel_spmd(_CACHE["crf"], maps, cores, trace=trace)
    LAST_RESULTS.append(res3)

    partition = np.concatenate(
        [np.asarray(r["part_out"])[0] for r in res3.results])
    emit = float(sum(np.asarray(r["emit_out"])[0, 0] for r in res3.results))

    # host-side numerator terms
    first_tag = labels[:, 0]
    last_tag = np.take_along_axis(labels, (lens - 1)[:, None], axis=1)[:, 0]
    tr_sc = float((trans[labels[:, :-1], labels[:, 1:]] * valid[:, 1:]).sum())
    host_num = float(start_t[first_tag].sum()) + tr_sc + float(end_t[last_tag].sum())

    loss = partition.sum() - emit - host_num
    return np.float32(loss)
